# revision 13
# baseline (speedup 1.0000x reference)
"""Trainium2 Bass kernel for nn_AdvancedTransformer_44942537785737.

6-layer post-LN transformer encoder, B=8, S=1024, D=1024, H=16, FF=4096.
Sharding: pure data-parallel — one sequence per NeuronCore, 8 cores, no
collectives.

Per-core layout: activations are kept feature-major ("transposed", [D, S])
resident in SBUF; weights stream from HBM in bf16. Matmuls run on the PE in
bf16 (fp32 accumulate); LayerNorm statistics use float32r matmuls against the
fp32 activation master for accuracy. Softmax is computed without max
subtraction (scores are bounded by design of the matmul scaling), with the
padding mask folded into the per-partition bias of the Exp activation and the
1/sum normalization folded in after the attention-value matmul via a
ones-column appended to V.
"""

import math
import sys

sys.path.insert(0, "/opt/trn_rl_repo")

import numpy as np
import ml_dtypes

P = 128
B = 8
S = 1024
D = 1024
H = 16
DK = 64
FF = 4096
NL = 6
NJ = S // 512          # 512-wide column chunks of the token dim
KD = D // P            # 8  k-tiles along D
MF = FF // P           # 32 m-tiles along FF
EPS_BLK = 1e-6
EPS_FIN = 1e-5

_CACHE = {}


def build_nc(n_layers=NL):
    import concourse.bass as bass
    import concourse.mybir as mybir
    import concourse.tile as tile
    from concourse import bacc
    from concourse.masks import make_identity
    from contextlib import ExitStack

    f32 = mybir.dt.float32
    bf16 = mybir.dt.bfloat16
    f32r = mybir.dt.float32r
    i32 = mybir.dt.int32
    AO = mybir.AluOpType
    AF = mybir.ActivationFunctionType

    # Pin the ACT table set: every activation we emit (Exp, Ln, Relu, Copy,
    # Identity) lives in natural_log_exp_and_others. Restricting the choice
    # to that set removes the ~2.6us Exp<->Ln table reload from every
    # LayerNorm critical path (2 reloads x 13 LNs otherwise).
    import concourse.hw_specs as hw_specs
    tabs = hw_specs.get_activation_tables("gen3")
    keep = "natural_log_exp_and_others"
    if keep in tabs:
        for name in tabs:
            if name != keep:
                tabs[name] = set()

    nc = bacc.Bacc(None, target_bir_lowering=False)

    d_tok = nc.dram_tensor("tok", [P, 8], i32, kind="ExternalInput")
    d_mask = nc.dram_tensor("maskb", [P, 8], f32, kind="ExternalInput")
    d_emb = nc.dram_tensor("emb32", [32000, D], f32, kind="ExternalInput")
    d_peT = nc.dram_tensor("peT", [P, KD, S], f32, kind="ExternalInput")
    d_wq = nc.dram_tensor("wq", [n_layers, KD, P, D], bf16, kind="ExternalInput")
    d_wk = nc.dram_tensor("wk", [n_layers, KD, P, D], bf16, kind="ExternalInput")
    d_wv = nc.dram_tensor("wv", [n_layers, KD, P, D], bf16, kind="ExternalInput")
    d_wo = nc.dram_tensor("wo", [n_layers, KD, P, D], bf16, kind="ExternalInput")
    d_w1 = nc.dram_tensor("w1", [n_layers, KD, P, FF], bf16, kind="ExternalInput")
    d_w2 = nc.dram_tensor("w2", [n_layers, MF, P, D], bf16, kind="ExternalInput")
    d_b1 = nc.dram_tensor("b1c", [n_layers, P, MF], f32, kind="ExternalInput")
    d_b2 = nc.dram_tensor("b2c", [n_layers, P, KD], f32, kind="ExternalInput")
    d_l1w = nc.dram_tensor("l1w", [n_layers, P, KD], f32, kind="ExternalInput")
    d_l1b = nc.dram_tensor("l1b", [n_layers, P, KD], f32, kind="ExternalInput")
    d_l2w = nc.dram_tensor("l2w", [n_layers, P, KD], f32, kind="ExternalInput")
    d_l2b = nc.dram_tensor("l2b", [n_layers, P, KD], f32, kind="ExternalInput")
    d_fnw = nc.dram_tensor("fnw", [P, KD], f32, kind="ExternalInput")
    d_fnb = nc.dram_tensor("fnb", [P, KD], f32, kind="ExternalInput")
    d_out = nc.dram_tensor("out", [S, D], f32, kind="ExternalOutput")

    with ExitStack() as ctx:
        tc = ctx.enter_context(tile.TileContext(nc))
        persist = ctx.enter_context(tc.tile_pool(name="persist", bufs=1))
        big = ctx.enter_context(tc.tile_pool(name="big", bufs=1))
        wpool = ctx.enter_context(tc.tile_pool(name="wpool", bufs=2))
        small = ctx.enter_context(tc.tile_pool(name="small", bufs=1))
        exppool = ctx.enter_context(tc.tile_pool(name="exppool", bufs=3))
        scratch = ctx.enter_context(tc.tile_pool(name="scratch", bufs=2))
        stat = ctx.enter_context(tc.tile_pool(name="stat", bufs=1))
        vecs = ctx.enter_context(tc.tile_pool(name="vecs", bufs=1))
        psum = ctx.enter_context(tc.tile_pool(name="psum", bufs=8, space="PSUM"))

        # --- persistent tiles ---
        x = persist.tile([P, KD, S], f32r)      # activation master, feature-major
        x_bf = persist.tile([P, KD, S], bf16)   # bf16 copy for matmul streaming
        QT = persist.tile([P, KD, S], bf16)
        KT = persist.tile([P, KD, S], bf16)
        Vaug = persist.tile([P, KD, H, DK + 1], bf16)  # token-major V + ones col
        ident = persist.tile([P, P], f32)
        ident_r = persist.tile([P, P], f32r)
        ones_c = persist.tile([P, 1], f32r)     # stats lhsT (holds 1/D)
        ones_cf = persist.tile([P, 1], f32)
        mask_sb = persist.tile([P, KD], f32)
        tok_sb = persist.tile([P, KD], i32)
        fnw_sb = persist.tile([P, KD], f32)
        fnb_sb = persist.tile([P, KD], f32)
        eps_blk = persist.tile([1, 1], f32)
        eps_fin = persist.tile([1, 1], f32)

        make_identity(nc, ident[:])
        nc.vector.tensor_copy(out=ident_r[:], in_=ident[:])
        nc.vector.memset(ones_cf[:], 1.0 / D)
        nc.vector.tensor_copy(out=ones_c[:], in_=ones_cf[:])
        nc.vector.memset(Vaug[:, :, :, DK:DK + 1], 1.0)
        nc.vector.memset(eps_blk[:], EPS_BLK)
        nc.vector.memset(eps_fin[:], EPS_FIN)
        nc.sync.dma_start(tok_sb[:], d_tok[:])
        nc.sync.dma_start(mask_sb[:], d_mask[:])
        nc.sync.dma_start(fnw_sb[:], d_fnw[:])
        nc.sync.dma_start(fnb_sb[:], d_fnb[:])

        def ps_tile():
            return psum.tile([P, 512], f32, tag="ps", name="ps")

        # =========================== embedding ===========================
        peT_sb = big.tile([P, KD, S], f32, tag="big")
        nc.sync.dma_start(peT_sb[:], d_peT[:])
        for t in range(KD):
            g = scratch.tile([P, S], f32, tag="scr")
            nc.gpsimd.indirect_dma_start(
                out=g[:],
                out_offset=None,
                in_=d_emb[:],
                in_offset=bass.IndirectOffsetOnAxis(ap=tok_sb[:, t:t + 1], axis=0),
            )
            for i in range(KD):
                pt = ps_tile()
                nc.tensor.transpose(pt[:, :P], g[:, i * P:(i + 1) * P], ident[:])
                nc.vector.tensor_tensor(
                    out=x[:, i, t * P:(t + 1) * P],
                    in0=pt[:, :P],
                    in1=peT_sb[:, i, t * P:(t + 1) * P],
                    op=AO.add,
                )
        for i in range(KD):
            nc.any.tensor_copy(out=x_bf[:, i, :], in_=x[:, i, :])

        # ============================ helpers ============================
        def layer_norm(w_col_tile, b_col_tile, eps_ap, cast_bf):
            """LayerNorm over features (partitions) of x, in place.

            Stats via float32r ones-matmuls; rstd = exp(-0.5*ln(var+eps)) so
            the ACT table set never leaves natural_log_exp.
            """
            ps_sx = [ps_tile() for _ in range(NJ)]
            ps_sq = [ps_tile() for _ in range(NJ)]
            ones_f = ones_c[:]
            for i in range(KD):
                xsq = scratch.tile([P, S], f32r, tag="scr", name="xsq")
                nc.vector.tensor_tensor(
                    out=xsq[:], in0=x[:, i, :], in1=x[:, i, :], op=AO.mult)
                for j in range(NJ):
                    nc.tensor.matmul(
                        ps_sx[j][0:1, :],
                        lhsT=ones_f,
                        rhs=x[:, i, j * 512:(j + 1) * 512],
                        start=(i == 0), stop=(i == KD - 1))
                    nc.tensor.matmul(
                        ps_sq[j][0:1, :],
                        lhsT=ones_f,
                        rhs=xsq[:, j * 512:(j + 1) * 512],
                        start=(i == 0), stop=(i == KD - 1))
            # ones_c holds 1/D, so psums are mean and E[x^2] directly
            mean_v = vecs.tile([1, S], f32, tag="mean_v")
            rstd_v = vecs.tile([1, S], f32, tag="rstd_v")
            for j in range(NJ):
                sl = slice(j * 512, (j + 1) * 512)
                nc.scalar.copy(mean_v[0:1, sl], ps_sx[j][0:1, :])
            # var = E[x^2] - mean^2 ; rstd = exp(-0.5 * ln(var + eps))
            nc.vector.tensor_tensor(
                out=rstd_v[:], in0=mean_v[:], in1=mean_v[:], op=AO.mult)
            for j in range(NJ):
                sl = slice(j * 512, (j + 1) * 512)
                nc.vector.tensor_tensor(
                    out=rstd_v[0:1, sl], in0=ps_sq[j][0:1, :],
                    in1=rstd_v[0:1, sl], op=AO.subtract)
            nc.scalar.activation(
                out=rstd_v[:], in_=rstd_v[:], func=AF.Ln,
                bias=eps_ap, scale=1.0)
            nc.scalar.activation(
                out=rstd_v[:], in_=rstd_v[:], func=AF.Exp, scale=-0.5)
            # broadcast mean/rstd over partitions on the (otherwise idle)
            # gpsimd engine so the PE never stalls on them
            mean_sb = stat.tile([P, S], f32, tag="mean_sb")
            rstd_sb = stat.tile([P, S], f32, tag="rstd_sb")
            nc.gpsimd.partition_broadcast(mean_sb[:, :], mean_v[0:1, :])
            nc.gpsimd.partition_broadcast(rstd_sb[:, :], rstd_v[0:1, :])
            for i in range(KD):
                nc.vector.tensor_tensor(
                    out=x[:, i, :], in0=x[:, i, :], in1=mean_sb[:], op=AO.subtract)
                nc.vector.tensor_tensor(
                    out=x[:, i, :], in0=x[:, i, :], in1=rstd_sb[:], op=AO.mult)
                nc.vector.tensor_scalar(
                    out=x[:, i, :], in0=x[:, i, :],
                    scalar1=w_col_tile[:, i:i + 1], scalar2=b_col_tile[:, i:i + 1],
                    op0=AO.mult, op1=AO.add)
                if cast_bf:
                    nc.any.tensor_copy(out=x_bf[:, i, :], in_=x[:, i, :])

        # ============================= layers ============================
        for l in range(n_layers):
            b1_sb = small.tile([P, MF], f32, tag="b1")
            b2_sb = small.tile([P, KD], f32, tag="b2")
            l1w_sb = small.tile([P, KD], f32, tag="l1w")
            l1b_sb = small.tile([P, KD], f32, tag="l1b")
            l2w_sb = small.tile([P, KD], f32, tag="l2w")
            l2b_sb = small.tile([P, KD], f32, tag="l2b")
            nc.sync.dma_start(b1_sb[:], d_b1[l])
            nc.sync.dma_start(b2_sb[:], d_b2[l])
            nc.sync.dma_start(l1w_sb[:], d_l1w[l])
            nc.sync.dma_start(l1b_sb[:], d_l1b[l])
            nc.sync.dma_start(l2w_sb[:], d_l2w[l])
            nc.sync.dma_start(l2b_sb[:], d_l2b[l])

            # ---- Q, K projections (feature-major out) ----
            for d_w, out_t, wtag in ((d_wq, QT, "wq"), (d_wk, KT, "wk")):
                for mg in range(2):
                    pts = [[ps_tile() for _ in range(NJ)] for _ in range(4)]
                    for k in range(KD):
                        wch = wpool.tile([P, 512], bf16, tag=wtag)
                        nc.sync.dma_start(
                            wch[:], d_w[l, k, :, mg * 512:(mg + 1) * 512])
                        for m4 in range(4):
                            for j in range(NJ):
                                nc.tensor.matmul(
                                    pts[m4][j][:, :],
                                    lhsT=wch[:, m4 * P:(m4 + 1) * P],
                                    rhs=x_bf[:, k, j * 512:(j + 1) * 512],
                                    start=(k == 0), stop=(k == KD - 1))
                    for m4 in range(4):
                        m = mg * 4 + m4
                        for j in range(NJ):
                            nc.any.tensor_copy(
                                out=out_t[:, m, j * 512:(j + 1) * 512],
                                in_=pts[m4][j][:, :])

            # ---- V projection (token-major out, heads strided with ones col) ----
            for jd in range(2):
                pts = [ps_tile() for _ in range(KD)]
                for k in range(KD):
                    wch = wpool.tile([P, 512], bf16, tag="wv")
                    nc.sync.dma_start(
                        wch[:], d_wv[l, k, :, jd * 512:(jd + 1) * 512])
                    for t in range(KD):
                        nc.tensor.matmul(
                            pts[t][:, :],
                            lhsT=x_bf[:, k, t * P:(t + 1) * P],
                            rhs=wch[:],
                            start=(k == 0), stop=(k == KD - 1))
                for t in range(KD):
                    src = pts[t][:, :].rearrange("p (h d) -> p h d", d=DK)
                    nc.any.tensor_copy(
                        out=Vaug[:, t, 8 * jd:8 * jd + 8, 0:DK], in_=src)

            # ---- attention, head by head ----
            ctxT = big.tile([P, KD, S], bf16, tag="big")
            pending = None

            def flush_head(h, C):
                ht, r0 = h // 2, (h % 2) * 64
                rv = vecs.tile([1, S], f32, tag="mean_v", name="rv")
                for j in range(NJ):
                    nc.vector.reciprocal(
                        out=rv[0:1, j * 512:(j + 1) * 512], in_=C[j][64:65, :])
                rb = stat.tile([64, S], f32, tag="mean_sb", name="rb")
                nc.gpsimd.partition_broadcast(rb[:, :], rv[0:1, :])
                if r0 == 0:
                    for j in range(NJ):
                        nc.vector.tensor_tensor(
                            out=ctxT[0:64, ht, j * 512:(j + 1) * 512],
                            in0=C[j][0:64, :],
                            in1=rb[:, j * 512:(j + 1) * 512],
                            op=AO.mult)
                else:
                    # normalized ctx lives on partitions 0:64 but must land on
                    # 64:128 — shift via SBUF->SBUF DMA
                    ctmp = exppool.tile([64, S], bf16, tag="exp", name="ctmp")
                    for j in range(NJ):
                        nc.vector.tensor_tensor(
                            out=ctmp[:, j * 512:(j + 1) * 512],
                            in0=C[j][0:64, :],
                            in1=rb[:, j * 512:(j + 1) * 512],
                            op=AO.mult)
                    nc.sync.dma_start(ctxT[64:128, ht, :], ctmp[:, :])

            for h in range(H):
                ht, r0 = h // 2, (h % 2) * 64
                C = [ps_tile() for _ in range(NJ)]
                prev_e = None
                for t in range(KD):
                    spt = [ps_tile() for _ in range(NJ)]
                    for j in range(NJ):
                        nc.tensor.matmul(
                            spt[j][:, :],
                            lhsT=KT[r0:r0 + 64, ht, t * P:(t + 1) * P],
                            rhs=QT[r0:r0 + 64, ht, j * 512:(j + 1) * 512],
                            start=True, stop=True)
                    e = exppool.tile([P, S], bf16, tag="exp")
                    for j in range(NJ):
                        nc.scalar.activation(
                            out=e[:, j * 512:(j + 1) * 512], in_=spt[j][:, :],
                            func=AF.Exp, bias=mask_sb[:, t:t + 1],
                            scale=1.0 / math.sqrt(DK))
                    if pending is not None:
                        flush_head(*pending)
                        pending = None
                    if prev_e is not None:
                        tp, ep = prev_e
                        for j in range(NJ):
                            nc.tensor.matmul(
                                C[j][0:DK + 1, :],
                                lhsT=Vaug[:, tp, h, 0:DK + 1],
                                rhs=ep[:, j * 512:(j + 1) * 512],
                                start=(tp == 0), stop=(tp == KD - 1))
                    prev_e = (t, e)
                tp, ep = prev_e
                for j in range(NJ):
                    nc.tensor.matmul(
                        C[j][0:DK + 1, :],
                        lhsT=Vaug[:, tp, h, 0:DK + 1],
                        rhs=ep[:, j * 512:(j + 1) * 512],
                        start=(tp == 0), stop=(tp == KD - 1))
                pending = (h, C)
            flush_head(*pending)
            pending = None

            # ---- attention output projection + residual ----
            for mg in range(2):
                pts = [[ps_tile() for _ in range(NJ)] for _ in range(4)]
                for k in range(KD):
                    wch = wpool.tile([P, 512], bf16, tag="wo")
                    nc.sync.dma_start(
                        wch[:], d_wo[l, k, :, mg * 512:(mg + 1) * 512])
                    for m4 in range(4):
                        for j in range(NJ):
                            nc.tensor.matmul(
                                pts[m4][j][:, :],
                                lhsT=wch[:, m4 * P:(m4 + 1) * P],
                                rhs=ctxT[:, k, j * 512:(j + 1) * 512],
                                start=(k == 0), stop=(k == KD - 1))
                for m4 in range(4):
                    m = mg * 4 + m4
                    for j in range(NJ):
                        nc.vector.tensor_tensor(
                            out=x[:, m, j * 512:(j + 1) * 512],
                            in0=x[:, m, j * 512:(j + 1) * 512],
                            in1=pts[m4][j][:, :], op=AO.add)

            layer_norm(l1w_sb, l1b_sb, eps_blk[0:1, :], cast_bf=True)

            # ---- FFN1: h1 = relu(x @ W1 + b1), feature-major [FF, S] ----
            h1T = big.tile([P, MF, S], bf16, tag="big")
            for mg in range(MF // 4):
                pts = [[ps_tile() for _ in range(NJ)] for _ in range(4)]
                for k in range(KD):
                    wch = wpool.tile([P, 512], bf16, tag="w1")
                    nc.sync.dma_start(
                        wch[:], d_w1[l, k, :, mg * 512:(mg + 1) * 512])
                    for m4 in range(4):
                        for j in range(NJ):
                            nc.tensor.matmul(
                                pts[m4][j][:, :],
                                lhsT=wch[:, m4 * P:(m4 + 1) * P],
                                rhs=x_bf[:, k, j * 512:(j + 1) * 512],
                                start=(k == 0), stop=(k == KD - 1))
                for m4 in range(4):
                    m = mg * 4 + m4
                    for j in range(NJ):
                        nc.scalar.activation(
                            out=h1T[:, m, j * 512:(j + 1) * 512],
                            in_=pts[m4][j][:, :], func=AF.Relu,
                            bias=b1_sb[:, m:m + 1], scale=1.0)

            # ---- FFN2 + residual ----
            for mg in range(2):
                pts = [[ps_tile() for _ in range(NJ)] for _ in range(4)]
                for k in range(MF):
                    wch = wpool.tile([P, 512], bf16, tag="w2")
                    nc.sync.dma_start(
                        wch[:], d_w2[l, k, :, mg * 512:(mg + 1) * 512])
                    for m4 in range(4):
                        for j in range(NJ):
                            nc.tensor.matmul(
                                pts[m4][j][:, :],
                                lhsT=wch[:, m4 * P:(m4 + 1) * P],
                                rhs=h1T[:, k, j * 512:(j + 1) * 512],
                                start=(k == 0), stop=(k == MF - 1))
                for m4 in range(4):
                    m = mg * 4 + m4
                    for j in range(NJ):
                        sl = slice(j * 512, (j + 1) * 512)
                        nc.vector.tensor_tensor(
                            out=x[:, m, sl], in0=x[:, m, sl],
                            in1=pts[m4][j][:, :], op=AO.add)
                    nc.vector.tensor_scalar_add(
                        out=x[:, m, :], in0=x[:, m, :],
                        scalar1=b2_sb[:, m:m + 1])

            layer_norm(l2w_sb, l2b_sb, eps_blk[0:1, :],
                       cast_bf=(l != n_layers - 1))

        # ===================== final LN + transpose out ==================
        layer_norm(fnw_sb, fnb_sb, eps_fin[0:1, :], cast_bf=False)
        for t in range(KD):
            ost = scratch.tile([P, S], f32, tag="scr")
            for i in range(KD):
                pt = psum.tile([P, 512], f32r, tag="ps", name="ps")
                nc.tensor.transpose(
                    pt[:, :P], x[:, i, t * P:(t + 1) * P], ident_r[:])
                nc.any.tensor_copy(
                    out=ost[:, i * P:(i + 1) * P], in_=pt[:, :P])
            nc.sync.dma_start(d_out[t * P:(t + 1) * P, :], ost[:])

    nc.finalize()
    return nc


def _prep_inputs(inputs, n_layers=NL):
    """Host-side rearrangement of the full model inputs into per-core maps."""
    bf = ml_dtypes.bfloat16
    tokens = np.asarray(inputs["tokens"], dtype=np.int32)
    emb = np.asarray(inputs["emb"], dtype=np.float32)

    emb32 = np.ascontiguousarray(emb * np.float32(math.sqrt(D)))

    def wsplit(w, ktiles):
        w = np.asarray(w, dtype=np.float32)[:n_layers]
        return np.ascontiguousarray(
            w.reshape(n_layers, ktiles, P, w.shape[-1]).astype(bf))

    wq = wsplit(inputs["Wq"], KD)
    wk = wsplit(inputs["Wk"], KD)
    wv = wsplit(inputs["Wv"], KD)
    wo = wsplit(inputs["Wo"], KD)
    w1 = wsplit(inputs["W1"], KD)
    w2 = wsplit(inputs["W2"], MF)

    def cols(v, n):
        v = np.asarray(v, dtype=np.float32)[:n_layers]
        return np.ascontiguousarray(v.reshape(n_layers, n, P).transpose(0, 2, 1))

    b1c = cols(inputs["b1"], MF)
    b2c = cols(inputs["b2"], KD)
    l1w = cols(inputs["ln1w"], KD)
    l1b = cols(inputs["ln1b"], KD)
    l2w = cols(inputs["ln2w"], KD)
    l2b = cols(inputs["ln2b"], KD)

    def fcols(v):
        v = np.asarray(v, dtype=np.float32)
        return np.ascontiguousarray(v.reshape(KD, P).T)

    fnw = fcols(inputs["fnw"])
    fnb = fcols(inputs["fnb"])

    pos = np.arange(S, dtype=np.float32)[:, None]
    div = np.exp(np.arange(0, D, 2, dtype=np.float32)
                 * np.float32(-math.log(10000.0) / D))
    ang = pos * div
    pe = np.stack([np.sin(ang), np.cos(ang)], axis=-1).reshape(S, D)
    peT = np.ascontiguousarray(
        pe.T.reshape(KD, P, S).transpose(1, 0, 2).astype(np.float32))

    shared = dict(emb32=emb32, peT=peT, wq=wq, wk=wk, wv=wv, wo=wo,
                  w1=w1, w2=w2, b1c=b1c, b2c=b2c, l1w=l1w, l1b=l1b,
                  l2w=l2w, l2b=l2b, fnw=fnw, fnb=fnb)
    in_maps = []
    for b in range(B):
        tok = np.ascontiguousarray(tokens[b].reshape(KD, P).T)
        maskb = np.ascontiguousarray(
            np.where(tokens[b] == 0, np.float32(-1e9),
                     np.float32(0.0)).reshape(KD, P).T)
        in_maps.append(dict(tok=tok, maskb=maskb, **shared))
    return in_maps


def run(inputs, n_layers=NL, trace=False, trace_kwargs=None):
    from concourse.bass_utils import run_bass_kernel_spmd

    key = n_layers
    if key not in _CACHE:
        _CACHE[key] = build_nc(n_layers)
    nc = _CACHE[key]
    in_maps = _prep_inputs(inputs, n_layers)
    kwargs = {}
    if trace:
        kwargs.update(trace=True, trace_kwargs=trace_kwargs or {})
    res = run_bass_kernel_spmd(nc, in_maps, core_ids=list(range(B)), **kwargs)
    out = np.stack([res.results[b]["out"] for b in range(B)], axis=0)
    return out, res


def kernel(**inputs):
    out, _ = run(inputs)
    return out


# revision 15
# speedup vs baseline: 1.1815x; 1.1815x over previous
"""Trainium2 Bass kernel for nn_AdvancedTransformer_44942537785737.

6-layer post-LN transformer encoder, B=8, S=1024, D=1024, H=16, FF=4096.
Sharding: pure data-parallel — one sequence per NeuronCore, 8 cores, no
collectives.

Per-core layout: activations are kept feature-major ("transposed", [D, S])
resident in SBUF; weights stream from HBM in bf16. Matmuls run on the PE in
bf16 (fp32 accumulate); LayerNorm statistics use float32r matmuls against the
fp32 activation master for accuracy. Softmax is computed without max
subtraction (scores are bounded by the 1/sqrt(dk) scaling of random-init
weights), with the padding mask folded into the per-partition bias of the Exp
activation and the 1/sum normalization folded in after the attention-value
matmul via a ones-column appended to V.

PSUM is managed as four [128, 1024] (two-bank) tiles; every matmul targets a
512-wide half so drains, softmax exp and reciprocal run as single wide
instructions.
"""

import math
import sys

sys.path.insert(0, "/opt/trn_rl_repo")

import numpy as np
import ml_dtypes

P = 128
B = 8
S = 1024
D = 1024
H = 16
DK = 64
FF = 4096
NL = 6
NJ = S // 512          # 512-wide column chunks of the token dim
KD = D // P            # 8  k-tiles along D
MF = FF // P           # 32 m-tiles along FF
EPS_BLK = 1e-6
EPS_FIN = 1e-5

_CACHE = {}


def build_nc(n_layers=NL):
    import concourse.bass as bass
    import concourse.mybir as mybir
    import concourse.tile as tile
    from concourse import bacc
    from concourse.masks import make_identity
    from contextlib import ExitStack

    f32 = mybir.dt.float32
    bf16 = mybir.dt.bfloat16
    f32r = mybir.dt.float32r
    i32 = mybir.dt.int32
    AO = mybir.AluOpType
    AF = mybir.ActivationFunctionType

    # Pin the ACT table set: every activation we emit (Exp, Ln, Relu, Copy,
    # Identity) lives in natural_log_exp_and_others. Restricting the choice
    # to that set removes the ~2.6us Exp<->Ln table reload from every
    # LayerNorm critical path.
    import concourse.hw_specs as hw_specs
    tabs = hw_specs.get_activation_tables("gen3")
    keep = "natural_log_exp_and_others"
    if keep in tabs:
        for name in tabs:
            if name != keep:
                tabs[name] = set()

    nc = bacc.Bacc(None, target_bir_lowering=False)

    d_tok = nc.dram_tensor("tok", [P, 8], i32, kind="ExternalInput")
    d_mask = nc.dram_tensor("maskb", [P, 8], f32, kind="ExternalInput")
    d_emb = nc.dram_tensor("emb32", [32000, D], f32, kind="ExternalInput")
    d_peT = nc.dram_tensor("peT", [P, KD, S], f32, kind="ExternalInput")
    d_wq = nc.dram_tensor("wq", [n_layers, KD, P, D], bf16, kind="ExternalInput")
    d_wk = nc.dram_tensor("wk", [n_layers, KD, P, D], bf16, kind="ExternalInput")
    d_wv = nc.dram_tensor("wv", [n_layers, KD, P, D], bf16, kind="ExternalInput")
    d_wo = nc.dram_tensor("wo", [n_layers, KD, P, D], bf16, kind="ExternalInput")
    d_w1 = nc.dram_tensor("w1", [n_layers, KD, P, FF], bf16, kind="ExternalInput")
    d_w2 = nc.dram_tensor("w2", [n_layers, MF, P, D], bf16, kind="ExternalInput")
    d_b1 = nc.dram_tensor("b1c", [n_layers, P, MF], f32, kind="ExternalInput")
    d_b2 = nc.dram_tensor("b2c", [n_layers, P, KD], f32, kind="ExternalInput")
    d_l1w = nc.dram_tensor("l1w", [n_layers, P, KD], f32, kind="ExternalInput")
    d_l1b = nc.dram_tensor("l1b", [n_layers, P, KD], f32, kind="ExternalInput")
    d_l2w = nc.dram_tensor("l2w", [n_layers, P, KD], f32, kind="ExternalInput")
    d_l2b = nc.dram_tensor("l2b", [n_layers, P, KD], f32, kind="ExternalInput")
    d_fnw = nc.dram_tensor("fnw", [P, KD], f32, kind="ExternalInput")
    d_fnb = nc.dram_tensor("fnb", [P, KD], f32, kind="ExternalInput")
    d_out = nc.dram_tensor("out", [S, D], f32, kind="ExternalOutput")

    with ExitStack() as ctx:
        tc = ctx.enter_context(tile.TileContext(nc))
        persist = ctx.enter_context(tc.tile_pool(name="persist", bufs=1))
        big = ctx.enter_context(tc.tile_pool(name="big", bufs=1))
        wpool = ctx.enter_context(tc.tile_pool(name="wpool", bufs=3))
        small = ctx.enter_context(tc.tile_pool(name="small", bufs=1))
        exppool = ctx.enter_context(tc.tile_pool(name="exppool", bufs=3))
        scratch = ctx.enter_context(tc.tile_pool(name="scratch", bufs=2))
        stat = ctx.enter_context(tc.tile_pool(name="stat", bufs=1))
        psum = ctx.enter_context(tc.tile_pool(name="psum", bufs=4, space="PSUM"))

        # --- persistent tiles ---
        x = persist.tile([P, KD, S], f32r)      # activation master, feature-major
        x_bf = persist.tile([P, KD, S], bf16)   # bf16 copy for matmul streaming
        QT = persist.tile([P, KD, S], bf16)
        KT = persist.tile([P, KD, S], bf16)
        Vaug = persist.tile([P, KD, H, DK + 1], bf16)  # token-major V + ones col
        ident = persist.tile([P, P], f32)
        ident_r = persist.tile([P, P], f32r)
        ones_c = persist.tile([P, 1], f32r)     # stats lhsT (holds 1/D)
        ones_cf = persist.tile([P, 1], f32)
        mask_sb = persist.tile([P, KD], f32)
        tok_sb = persist.tile([P, KD], i32)
        fnw_sb = persist.tile([P, KD], f32)
        fnb_sb = persist.tile([P, KD], f32)
        eps_blk = persist.tile([1, 1], f32)
        eps_fin = persist.tile([1, 1], f32)

        make_identity(nc, ident[:])
        nc.vector.tensor_copy(out=ident_r[:], in_=ident[:])
        nc.vector.memset(ones_cf[:], 1.0 / D)
        nc.vector.tensor_copy(out=ones_c[:], in_=ones_cf[:])
        nc.vector.memset(Vaug[:, :, :, DK:DK + 1], 1.0)
        nc.vector.memset(eps_blk[:], EPS_BLK)
        nc.vector.memset(eps_fin[:], EPS_FIN)
        nc.sync.dma_start(tok_sb[:], d_tok[:])
        nc.sync.dma_start(mask_sb[:], d_mask[:])
        nc.sync.dma_start(fnw_sb[:], d_fnw[:])
        nc.sync.dma_start(fnb_sb[:], d_fnb[:])

        def ps_tile(dtype=f32):
            return psum.tile([P, S], dtype, tag="ps", name="ps")

        def jsl(j):
            return slice(j * 512, (j + 1) * 512)

        # =========================== embedding ===========================
        peT_sb = big.tile([P, KD, S], f32, tag="big")
        nc.sync.dma_start(peT_sb[:], d_peT[:])
        for t in range(KD):
            g = scratch.tile([P, S], f32, tag="scr")
            nc.gpsimd.indirect_dma_start(
                out=g[:],
                out_offset=None,
                in_=d_emb[:],
                in_offset=bass.IndirectOffsetOnAxis(ap=tok_sb[:, t:t + 1], axis=0),
            )
            for ii in range(0, KD, 2):
                pt = ps_tile()
                for h2 in range(2):
                    i = ii + h2
                    nc.tensor.transpose(
                        pt[:, h2 * 512:h2 * 512 + P],
                        g[:, i * P:(i + 1) * P], ident[:])
                    nc.vector.tensor_tensor(
                        out=x[:, i, t * P:(t + 1) * P],
                        in0=pt[:, h2 * 512:h2 * 512 + P],
                        in1=peT_sb[:, i, t * P:(t + 1) * P],
                        op=AO.add,
                    )
        for i in range(KD):
            nc.any.tensor_copy(out=x_bf[:, i, :], in_=x[:, i, :])

        # ============================ helpers ============================
        def layer_norm(w_col_tile, b_col_tile, eps_ap, cast_bf):
            """LayerNorm over features (partitions) of x, in place.

            Stats via float32r ones-matmuls into psum; rstd =
            exp(-0.5*ln(var+eps)); mean/rstd broadcast across partitions on
            gpsimd (in place, from row 0 of the broadcast target).
            """
            ps_sx = ps_tile()
            ps_sq = ps_tile()
            for i in range(KD):
                xsq = scratch.tile([P, S], f32r, tag="scr", name="xsq")
                nc.vector.tensor_tensor(
                    out=xsq[:], in0=x[:, i, :], in1=x[:, i, :], op=AO.mult)
                for j in range(NJ):
                    nc.tensor.matmul(
                        ps_sx[0:1, jsl(j)], lhsT=ones_c[:], rhs=x[:, i, jsl(j)],
                        start=(i == 0), stop=(i == KD - 1))
                    nc.tensor.matmul(
                        ps_sq[0:1, jsl(j)], lhsT=ones_c[:], rhs=xsq[:, jsl(j)],
                        start=(i == 0), stop=(i == KD - 1))
            # vectors live in row 0 of their broadcast targets
            mean_sb = stat.tile([P, S], f32, tag="mean_sb")
            rstd_sb = stat.tile([P, S], f32, tag="rstd_sb")
            mean_v = mean_sb[0:1, :]
            rstd_v = rstd_sb[0:1, :]
            nc.scalar.copy(mean_v, ps_sx[0:1, :])
            # var = E[x^2] - mean^2 ; rstd = exp(-0.5 * ln(var + eps))
            nc.vector.tensor_tensor(
                out=rstd_v, in0=mean_v, in1=mean_v, op=AO.mult)
            nc.vector.tensor_tensor(
                out=rstd_v, in0=ps_sq[0:1, :], in1=rstd_v, op=AO.subtract)
            nc.scalar.activation(
                out=rstd_v, in_=rstd_v, func=AF.Ln, bias=eps_ap, scale=1.0)
            nc.scalar.activation(
                out=rstd_v, in_=rstd_v, func=AF.Exp, scale=-0.5)
            nc.gpsimd.partition_broadcast(mean_sb[:, :], mean_v)
            nc.gpsimd.partition_broadcast(rstd_sb[:, :], rstd_v)
            for i in range(KD):
                nc.vector.tensor_tensor(
                    out=x[:, i, :], in0=x[:, i, :], in1=mean_sb[:], op=AO.subtract)
                nc.vector.tensor_tensor(
                    out=x[:, i, :], in0=x[:, i, :], in1=rstd_sb[:], op=AO.mult)
                nc.scalar.activation(
                    out=x[:, i, :], in_=x[:, i, :], func=AF.Identity,
                    bias=b_col_tile[:, i:i + 1], scale=w_col_tile[:, i:i + 1])
                if cast_bf:
                    nc.any.tensor_copy(out=x_bf[:, i, :], in_=x[:, i, :])

        # ============================= layers ============================
        for l in range(n_layers):
            b1_sb = small.tile([P, MF], f32, tag="b1")
            b2_sb = small.tile([P, KD], f32, tag="b2")
            l1w_sb = small.tile([P, KD], f32, tag="l1w")
            l1b_sb = small.tile([P, KD], f32, tag="l1b")
            l2w_sb = small.tile([P, KD], f32, tag="l2w")
            l2b_sb = small.tile([P, KD], f32, tag="l2b")
            nc.sync.dma_start(b1_sb[:], d_b1[l])
            nc.sync.dma_start(b2_sb[:], d_b2[l])
            nc.sync.dma_start(l1w_sb[:], d_l1w[l])
            nc.sync.dma_start(l1b_sb[:], d_l1b[l])
            nc.sync.dma_start(l2w_sb[:], d_l2w[l])
            nc.sync.dma_start(l2b_sb[:], d_l2b[l])

            # ---- Q, K projections (feature-major out) ----
            for d_w, out_t, wtag in ((d_wq, QT, "wq"), (d_wk, KT, "wk")):
                for mg in range(2):
                    pts = [ps_tile() for _ in range(4)]
                    for k in range(KD):
                        wch = wpool.tile([P, 512], bf16, tag=wtag)
                        nc.sync.dma_start(
                            wch[:], d_w[l, k, :, mg * 512:(mg + 1) * 512])
                        for m4 in range(4):
                            for j in range(NJ):
                                nc.tensor.matmul(
                                    pts[m4][:, jsl(j)],
                                    lhsT=wch[:, m4 * P:(m4 + 1) * P],
                                    rhs=x_bf[:, k, jsl(j)],
                                    start=(k == 0), stop=(k == KD - 1))
                    for m4 in range(4):
                        nc.any.tensor_copy(
                            out=out_t[:, mg * 4 + m4, :], in_=pts[m4][:, :])

            # ---- V projection (token-major out, heads strided, ones col) ----
            for jd in range(2):
                pts = [ps_tile() for _ in range(4)]
                for k in range(KD):
                    wch = wpool.tile([P, 512], bf16, tag="wv")
                    nc.sync.dma_start(
                        wch[:], d_wv[l, k, :, jd * 512:(jd + 1) * 512])
                    for t in range(KD):
                        nc.tensor.matmul(
                            pts[t // 2][:, jsl(t % 2)],
                            lhsT=x_bf[:, k, t * P:(t + 1) * P],
                            rhs=wch[:],
                            start=(k == 0), stop=(k == KD - 1))
                for t in range(KD):
                    src = pts[t // 2][:, jsl(t % 2)].rearrange(
                        "p (h d) -> p h d", d=DK)
                    nc.any.tensor_copy(
                        out=Vaug[:, t, 8 * jd:8 * jd + 8, 0:DK], in_=src)

            # ---- attention, head by head ----
            ctxT = big.tile([P, KD, S], bf16, tag="big")
            pending = None

            def flush_head(h, C):
                ht, r0 = h // 2, (h % 2) * 64
                rb = stat.tile([64, S], f32, tag="mean_sb", name="rb")
                nc.vector.reciprocal(out=rb[0:1, :], in_=C[64:65, :])
                nc.gpsimd.partition_broadcast(rb[:, :], rb[0:1, :])
                if r0 == 0:
                    nc.vector.tensor_tensor(
                        out=ctxT[0:64, ht, :], in0=C[0:64, :], in1=rb[:, :],
                        op=AO.mult)
                else:
                    # normalized ctx computed on partitions 0:64 must land on
                    # 64:128 — shift via SBUF->SBUF DMA
                    ctmp = exppool.tile([64, S], bf16, tag="exp", name="ctmp")
                    nc.vector.tensor_tensor(
                        out=ctmp[:, :], in0=C[0:64, :], in1=rb[:, :], op=AO.mult)
                    nc.sync.dma_start(ctxT[64:128, ht, :], ctmp[:, :])

            for h in range(H):
                ht, r0 = h // 2, (h % 2) * 64
                C = ps_tile()
                prev_e = None
                for t in range(KD):
                    spt = ps_tile()
                    for j in range(NJ):
                        nc.tensor.matmul(
                            spt[:, jsl(j)],
                            lhsT=KT[r0:r0 + 64, ht, t * P:(t + 1) * P],
                            rhs=QT[r0:r0 + 64, ht, jsl(j)],
                            start=True, stop=True)
                    e = exppool.tile([P, S], bf16, tag="exp")
                    nc.scalar.activation(
                        out=e[:, :], in_=spt[:, :], func=AF.Exp,
                        bias=mask_sb[:, t:t + 1], scale=1.0 / math.sqrt(DK))
                    if pending is not None:
                        flush_head(*pending)
                        pending = None
                    if prev_e is not None:
                        tp, ep = prev_e
                        for j in range(NJ):
                            nc.tensor.matmul(
                                C[0:DK + 1, jsl(j)],
                                lhsT=Vaug[:, tp, h, 0:DK + 1],
                                rhs=ep[:, jsl(j)],
                                start=(tp == 0), stop=(tp == KD - 1))
                    prev_e = (t, e)
                tp, ep = prev_e
                for j in range(NJ):
                    nc.tensor.matmul(
                        C[0:DK + 1, jsl(j)],
                        lhsT=Vaug[:, tp, h, 0:DK + 1],
                        rhs=ep[:, jsl(j)],
                        start=(tp == 0), stop=(tp == KD - 1))
                pending = (h, C)
            flush_head(*pending)
            pending = None

            # ---- attention output projection + residual ----
            for mg in range(2):
                pts = [ps_tile() for _ in range(4)]
                for k in range(KD):
                    wch = wpool.tile([P, 512], bf16, tag="wo")
                    nc.sync.dma_start(
                        wch[:], d_wo[l, k, :, mg * 512:(mg + 1) * 512])
                    for m4 in range(4):
                        for j in range(NJ):
                            nc.tensor.matmul(
                                pts[m4][:, jsl(j)],
                                lhsT=wch[:, m4 * P:(m4 + 1) * P],
                                rhs=ctxT[:, k, jsl(j)],
                                start=(k == 0), stop=(k == KD - 1))
                for m4 in range(4):
                    m = mg * 4 + m4
                    nc.vector.tensor_tensor(
                        out=x[:, m, :], in0=x[:, m, :], in1=pts[m4][:, :],
                        op=AO.add)

            layer_norm(l1w_sb, l1b_sb, eps_blk[0:1, :], cast_bf=True)

            # ---- FFN1: h1 = relu(x @ W1 + b1), feature-major [FF, S] ----
            h1T = big.tile([P, MF, S], bf16, tag="big")
            for mg in range(MF // 4):
                pts = [ps_tile() for _ in range(4)]
                for k in range(KD):
                    wch = wpool.tile([P, 512], bf16, tag="w1")
                    nc.sync.dma_start(
                        wch[:], d_w1[l, k, :, mg * 512:(mg + 1) * 512])
                    for m4 in range(4):
                        for j in range(NJ):
                            nc.tensor.matmul(
                                pts[m4][:, jsl(j)],
                                lhsT=wch[:, m4 * P:(m4 + 1) * P],
                                rhs=x_bf[:, k, jsl(j)],
                                start=(k == 0), stop=(k == KD - 1))
                for m4 in range(4):
                    m = mg * 4 + m4
                    nc.scalar.activation(
                        out=h1T[:, m, :], in_=pts[m4][:, :], func=AF.Relu,
                        bias=b1_sb[:, m:m + 1], scale=1.0)

            # ---- FFN2 + residual ----
            for mg in range(2):
                pts = [ps_tile() for _ in range(4)]
                for k in range(MF):
                    wch = wpool.tile([P, 512], bf16, tag="w2")
                    nc.sync.dma_start(
                        wch[:], d_w2[l, k, :, mg * 512:(mg + 1) * 512])
                    for m4 in range(4):
                        for j in range(NJ):
                            nc.tensor.matmul(
                                pts[m4][:, jsl(j)],
                                lhsT=wch[:, m4 * P:(m4 + 1) * P],
                                rhs=h1T[:, k, jsl(j)],
                                start=(k == 0), stop=(k == MF - 1))
                for m4 in range(4):
                    m = mg * 4 + m4
                    nc.vector.tensor_tensor(
                        out=x[:, m, :], in0=x[:, m, :], in1=pts[m4][:, :],
                        op=AO.add)
                    nc.vector.tensor_scalar_add(
                        out=x[:, m, :], in0=x[:, m, :],
                        scalar1=b2_sb[:, m:m + 1])

            layer_norm(l2w_sb, l2b_sb, eps_blk[0:1, :],
                       cast_bf=(l != n_layers - 1))

        # ===================== final LN + transpose out ==================
        layer_norm(fnw_sb, fnb_sb, eps_fin[0:1, :], cast_bf=False)
        for t in range(KD):
            ost = scratch.tile([P, S], f32, tag="scr")
            for ii in range(0, KD, 2):
                pt = ps_tile(f32r)
                for h2 in range(2):
                    i = ii + h2
                    nc.tensor.transpose(
                        pt[:, h2 * 512:h2 * 512 + P],
                        x[:, i, t * P:(t + 1) * P], ident_r[:])
                    nc.any.tensor_copy(
                        out=ost[:, i * P:(i + 1) * P],
                        in_=pt[:, h2 * 512:h2 * 512 + P])
            nc.sync.dma_start(d_out[t * P:(t + 1) * P, :], ost[:])

    nc.finalize()
    return nc


def _prep_inputs(inputs, n_layers=NL):
    """Host-side rearrangement of the full model inputs into per-core maps."""
    bf = ml_dtypes.bfloat16
    tokens = np.asarray(inputs["tokens"], dtype=np.int32)
    emb = np.asarray(inputs["emb"], dtype=np.float32)

    emb32 = np.ascontiguousarray(emb * np.float32(math.sqrt(D)))

    def wsplit(w, ktiles):
        w = np.asarray(w, dtype=np.float32)[:n_layers]
        return np.ascontiguousarray(
            w.reshape(n_layers, ktiles, P, w.shape[-1]).astype(bf))

    wq = wsplit(inputs["Wq"], KD)
    wk = wsplit(inputs["Wk"], KD)
    wv = wsplit(inputs["Wv"], KD)
    wo = wsplit(inputs["Wo"], KD)
    w1 = wsplit(inputs["W1"], KD)
    w2 = wsplit(inputs["W2"], MF)

    def cols(v, n):
        v = np.asarray(v, dtype=np.float32)[:n_layers]
        return np.ascontiguousarray(v.reshape(n_layers, n, P).transpose(0, 2, 1))

    b1c = cols(inputs["b1"], MF)
    b2c = cols(inputs["b2"], KD)
    l1w = cols(inputs["ln1w"], KD)
    l1b = cols(inputs["ln1b"], KD)
    l2w = cols(inputs["ln2w"], KD)
    l2b = cols(inputs["ln2b"], KD)

    def fcols(v):
        v = np.asarray(v, dtype=np.float32)
        return np.ascontiguousarray(v.reshape(KD, P).T)

    fnw = fcols(inputs["fnw"])
    fnb = fcols(inputs["fnb"])

    pos = np.arange(S, dtype=np.float32)[:, None]
    div = np.exp(np.arange(0, D, 2, dtype=np.float32)
                 * np.float32(-math.log(10000.0) / D))
    ang = pos * div
    pe = np.stack([np.sin(ang), np.cos(ang)], axis=-1).reshape(S, D)
    peT = np.ascontiguousarray(
        pe.T.reshape(KD, P, S).transpose(1, 0, 2).astype(np.float32))

    shared = dict(emb32=emb32, peT=peT, wq=wq, wk=wk, wv=wv, wo=wo,
                  w1=w1, w2=w2, b1c=b1c, b2c=b2c, l1w=l1w, l1b=l1b,
                  l2w=l2w, l2b=l2b, fnw=fnw, fnb=fnb)
    in_maps = []
    for b in range(B):
        tok = np.ascontiguousarray(tokens[b].reshape(KD, P).T)
        maskb = np.ascontiguousarray(
            np.where(tokens[b] == 0, np.float32(-1e9),
                     np.float32(0.0)).reshape(KD, P).T)
        in_maps.append(dict(tok=tok, maskb=maskb, **shared))
    return in_maps


def run(inputs, n_layers=NL, trace=False, trace_kwargs=None):
    from concourse.bass_utils import run_bass_kernel_spmd

    key = n_layers
    if key not in _CACHE:
        _CACHE[key] = build_nc(n_layers)
    nc = _CACHE[key]
    in_maps = _prep_inputs(inputs, n_layers)
    kwargs = {}
    if trace:
        kwargs.update(trace=True, trace_kwargs=trace_kwargs or {})
    res = run_bass_kernel_spmd(nc, in_maps, core_ids=list(range(B)), **kwargs)
    out = np.stack([res.results[b]["out"] for b in range(B)], axis=0)
    return out, res


def kernel(**inputs):
    out, _ = run(inputs)
    return out
